# revision 5
# baseline (speedup 1.0000x reference)
import sys
sys.path.insert(0, '/opt/trn_rl_repo')
import numpy as np
import ml_dtypes
from concurrent.futures import ThreadPoolExecutor
from contextlib import ExitStack

import concourse.bass as bass
import concourse.mybir as mybir
from concourse.bass_utils import run_bass_kernel_spmd

# Problem: y[b,s,o] = x[b]@W.T + bias + (x[b]@a[idx[b]].T)@b[idx[b]].T
# B=8 batch elements -> data-parallel, one per NeuronCore.
B, S, D, RANK = 8, 2048, 4096, 16
P = 128

_BF = mybir.dt.bfloat16
_F32 = mybir.dt.float32


def build_nc(s=S, d=D, warm=True):
    KT = d // P          # contraction tiles
    NQ = 4               # s-quarters
    SQ = s // NQ
    NJ = d // 512        # o-blocks: OJ=512 fills one PSUM bank exactly
    OJ = d // NJ
    NT = SQ // P         # s-tiles per quarter
    XC = 4               # x DMA chunks per quarter
    KC = KT // XC
    WC = 2               # w DMA chunks per o-block
    KW = KT // WC
    NWARM = 42           # PE warmup matmuls: bridge the DMA bootstrap window
                         # so HAM stays warm into the first real matmuls
    KH = KC // 2         # sub-chunk of the very first x chunk
    KWH = KW // 2        # sub-chunk of the very first w chunk

    nc = bass.Bass()
    xt = nc.declare_dram_parameter("xt", [d, s], _BF, isOutput=False)
    wt = nc.declare_dram_parameter("wt", [d, d], _BF, isOutput=False)
    at = nc.declare_dram_parameter("at", [d, RANK], _BF, isOutput=False)
    bt = nc.declare_dram_parameter("bt", [2 * RANK, d], _BF, isOutput=False)
    ones = nc.declare_dram_parameter("ones", [RANK, s], _BF, isOutput=False)
    y = nc.declare_dram_parameter("y", [s, d], _BF, isOutput=True)

    xt_t = xt.rearrange("(k p) s -> p k s", p=P)
    wt_t = wt.rearrange("(k p) o -> p k o", p=P)
    at_t = at.rearrange("(k p) r -> p k r", p=P)

    with (
        nc.sbuf_tensor([P, 2, KT, SQ], _BF) as x_sb,
        nc.sbuf_tensor([P, 2, KT, OJ], _BF) as w_sb,
        nc.sbuf_tensor([P, KT, RANK], _BF) as at_sb,
        nc.sbuf_tensor([P, d], _BF) as bt_sb,
        nc.sbuf_tensor([P, s], _BF) as inter_sb,
        nc.sbuf_tensor([P, OJ + P], _BF) as scratch_sb,
        nc.sbuf_tensor([P, 4, OJ], _BF) as out_sb,
        nc.psum_tensor([P, 7, OJ], _F32) as psum_y,
        nc.psum_tensor([P, SQ], _F32) as psum_i,
        ExitStack() as _sems,
        nc.Block() as block,
    ):
        def _sem(name):
            return _sems.enter_context(nc.semaphore(name))

        # One semaphore per (chunk lane, double-buffer parity): at any wait
        # point only one DMA can be outstanding on a given semaphore, so
        # out-of-order per-engine completions can't satisfy a wait early.
        x_sems = [[_sem(f"x_sem{c}_{p}") for p in range(2)] for c in range(XC)]
        w_sems = [[_sem(f"w_sem{h}_{p}") for p in range(2)] for h in range(WC)]
        c_sem = _sem("c_sem")
        pe_sem = _sem("pe_sem")
        pei_sem = _sem("pei_sem")
        dve_sem = _sem("dve_sem")
        ev_sem = _sem("ev_sem")
        st_sems = [_sem(f"st_sem{p}") for p in range(4)]
        fin_sem = _sem("fin_sem")
        # dedicated sems for the extra-fine first x/w sub-chunks (quarter 0,
        # j-block 0 only) — each has exactly one DMA, so waits are race-free
        xa_sem = _sem("xa_sem")
        xb_sem = _sem("xb_sem")
        wa_sem = _sem("wa_sem")
        wb_sem = _sem("wb_sem")
        xc1a_sem = _sem("xc1a_sem")
        xc1b_sem = _sem("xc1b_sem")
        xc2a_sem = _sem("xc2a_sem")
        xc2b_sem = _sem("xc2b_sem")
        xc3a_sem = _sem("xc3a_sem")
        xc3b_sem = _sem("xc3b_sem")
        w1a_sem = _sem("w1a_sem")
        w1b_sem = _sem("w1b_sem")
        cb_sem = _sem("cb_sem")    # bt+ones: only gate the LoRA matmuls
        def _dma_x(eng, q):
            for c in range(XC):
                eng.dma_start(
                    x_sb[:, q % 2, c * KC:(c + 1) * KC, :],
                    xt_t[:, c * KC:(c + 1) * KC, q * SQ:(q + 1) * SQ],
                ).then_inc(x_sems[c][q % 2], 16)

        def _dma_w(eng, wj, h):
            j = wj % NJ
            eng.dma_start(
                w_sb[:, j % 2, h * KW:(h + 1) * KW, :],
                wt_t[:, h * KW:(h + 1) * KW, j * OJ:(j + 1) * OJ],
            ).then_inc(w_sems[h][j % 2], 16)

        def _dma_xc(eng, q, c):
            eng.dma_start(
                x_sb[:, q % 2, c * KC:(c + 1) * KC, :],
                xt_t[:, c * KC:(c + 1) * KC, q * SQ:(q + 1) * SQ],
            ).then_inc(x_sems[c][q % 2], 16)

        def _dma_xh(sync, k0, k1, sem):
            sync.dma_start(
                x_sb[:, 0, k0:k1, :], xt_t[:, k0:k1, 0:SQ]
            ).then_inc(sem, 16)

        @block.sync
        def _(sync):
            sync.dma_start(at_sb[:], at_t).then_inc(c_sem, 16)
            # Interleave x-chunks and w-chunks of the first j-block so the
            # chunk-paced first groups can start matmuls as early as possible;
            # the leading x/w chunks are split extra fine so the very first
            # matmuls start ~6us earlier.
            _dma_xh(sync, 0, KH, xa_sem)
            sync.dma_start(
                w_sb[:, 0, 0:KWH, :], wt_t[:, 0:KWH, 0:OJ]
            ).then_inc(wa_sem, 16)
            _dma_xh(sync, KH, KC, xb_sem)
            sync.dma_start(
                w_sb[:, 0, KWH:KW, :], wt_t[:, KWH:KW, 0:OJ]
            ).then_inc(wb_sem, 16)
            _dma_xh(sync, KC, KC + KH, xc1a_sem)
            _dma_xh(sync, KC + KH, 2 * KC, xc1b_sem)
            sync.dma_start(
                w_sb[:, 0, KW:KW + KWH, :], wt_t[:, KW:KW + KWH, 0:OJ]
            ).then_inc(w1a_sem, 16)
            _dma_xh(sync, KW, KW + KH, xc2a_sem)
            _dma_xh(sync, KW + KH, KW + KC, xc2b_sem)
            sync.dma_start(
                w_sb[:, 0, KW + KWH:KT, :], wt_t[:, KW + KWH:KT, 0:OJ]
            ).then_inc(w1b_sem, 16)
            _dma_xh(sync, KW + KC, KW + KC + KH, xc3a_sem)
            _dma_xh(sync, KW + KC + KH, KT, xc3b_sem)
            # first chunk of w j1 before the (lora-only) bt/ones constants,
            # so j1's base matmuls aren't delayed behind them
            _dma_w(sync, 1, 0)
            sync.dma_start(bt_sb[0:2 * RANK, :], bt[:, :]).then_inc(cb_sem, 16)
            sync.dma_start(inter_sb[RANK:2 * RANK, :], ones[:, :]).then_inc(
                cb_sem, 16
            )
            _dma_w(sync, 1, 1)
            for q in range(NQ):
                if q >= 2:
                    sync.wait_ge(ev_sem, NJ * NT * (q - 1))
                if q > 0:
                    _dma_x(sync, q)
                for j in range(2 if q == 0 else 0, NJ):
                    wj = q * NJ + j
                    if wj >= 2:
                        sync.wait_ge(ev_sem, NT * (wj - 1))
                    for h in range(WC):
                        _dma_w(sync, wj, h)
            # Final stores go through this idle HW-DGE queue: the gpsimd
            # SW-DGE ring then drains ~3us earlier, shortening the tail.
            for g in range(NQ * NJ * NT - 4, NQ * NJ * NT):
                q, rem = divmod(g, NJ * NT)
                j, t = divmod(rem, NT)
                st = q * NT + t
                sync.wait_ge(ev_sem, g + 1)
                # fin_sem is HW-DGE-only (st_sems belong to the gpsimd SW-DGE
                # ring); nothing waits on it — the epilogue DRAIN guarantees
                # completion.
                sync.dma_start(
                    y[st * P:(st + 1) * P, j * OJ:(j + 1) * OJ], out_sb[:, g % 4, :]
                ).then_inc(fin_sem, 16)

        @block.tensor
        def _(tensor):
            # Warm the PE (HAM un-throttle) on scratch data while the first
            # input DMAs are still in flight.
            for _ in range(NWARM if warm else 0):
                nc.tensor.matmul(
                    psum_y[:, 0, :], scratch_sb[:, OJ:OJ + P], scratch_sb[:, 0:OJ],
                    start=True, stop=True,
                )

            _xq0 = [(xa_sem, xb_sem), (xc1a_sem, xc1b_sem),
                    (xc2a_sem, xc2b_sem), (xc3a_sem, xc3b_sem)]

            def _x_wait(q, c):
                if q == 0:
                    tensor.wait_ge(_xq0[c][0], 16)
                    tensor.wait_ge(_xq0[c][1], 16)
                    return
                # quarter 0 is entirely off the x_sems lanes
                tensor.wait_ge(x_sems[c][q % 2], 16 * ((q + 1) // 2))

            def _w_wait(wj, h):
                j = wj % NJ
                th = 16 * (wj // 2 + 1)
                if j % 2 == 0:
                    th -= 16   # block (0,0) moved off the even-parity lanes
                tensor.wait_ge(w_sems[h][j % 2], th)

            def _inter(q):
                mm = None
                for c in range(XC):
                    _x_wait(q, c)
                    for i in range(c * KC, (c + 1) * KC):
                        mm = nc.tensor.matmul(
                            psum_i[0:RANK, :], at_sb[:, i, :], x_sb[:, q % 2, i, :],
                            start=(i == 0), stop=(i == KT - 1),
                        )
                mm.then_inc(pei_sem, 1)

            # Quarter 0, j-block 0: run the base matmuls FIRST, paced chunk by
            # chunk as x/w interleave on the DMA queue — real work starts
            # ~10us earlier than waiting for the full quarter. The LoRA
            # stop-matmuls for these groups are backfilled after `inter` is
            # computed; their accumulation groups (banks 0..NT-1) stay open
            # meanwhile.
            _ranges = [
                (0, KH, [(xa_sem, 16), (wa_sem, 16)]),
                (KH, KC, [(xb_sem, 16)]),
                (KC, KC + KH, [(xc1a_sem, 16), (wb_sem, 16)]),
                (KC + KH, KW, [(xc1b_sem, 16)]),
                (KW, KW + KH, [(xc2a_sem, 16), (w1a_sem, 16)]),
                (KW + KH, KW + KC, [(xc2b_sem, 16)]),
                (KW + KC, KW + KC + KH, [(xc3a_sem, 16), (w1b_sem, 16)]),
                (KW + KC + KH, KT, [(xc3b_sem, 16)]),
            ]
            for lo, hi, waits in _ranges:
                for sem, th in waits:
                    tensor.wait_ge(sem, th)
                for t in range(NT):
                    for i in range(lo, hi):
                        nc.tensor.matmul(
                            psum_y[:, t, :],
                            x_sb[:, 0, i, t * P:(t + 1) * P],
                            w_sb[:, 0, i, :],
                            start=(i == 0), stop=False,
                        )
            tensor.wait_ge(c_sem, 112)  # at DMA + 6 memsets
            _inter(0)
            tensor.wait_ge(cb_sem, 32)  # bt + ones, needed by LoRA mms only
            tensor.wait_ge(dve_sem, 1)
            for t in range(NT):
                nc.tensor.matmul(
                    psum_y[:, t, :],
                    inter_sb[:, t * P:(t + 1) * P],
                    bt_sb[:, 0:OJ],
                    start=False, stop=True,
                ).then_inc(pe_sem, 1)

            g = NT
            for q in range(NQ):
                if q > 0:
                    tensor.wait_ge(dve_sem, q)     # psum_i WAR
                    _inter(q)
                for j in range(1 if q == 0 else 0, NJ):
                    wj = q * NJ + j
                    _w_wait(wj, 0)
                    need_h2 = True
                    for t in range(NT):
                        st = q * NT + t
                        if g >= 7:
                            tensor.wait_ge(ev_sem, g - 6)
                        for i in range(KW):
                            nc.tensor.matmul(
                                psum_y[:, g % 7, :],
                                x_sb[:, q % 2, i, t * P:(t + 1) * P],
                                w_sb[:, j % 2, i, :],
                                start=(i == 0), stop=False,
                            )
                        if need_h2:
                            _w_wait(wj, 1)
                            need_h2 = False
                        for i in range(KW, KT):
                            nc.tensor.matmul(
                                psum_y[:, g % 7, :],
                                x_sb[:, q % 2, i, t * P:(t + 1) * P],
                                w_sb[:, j % 2, i, :],
                                start=False, stop=False,
                            )
                        tensor.wait_ge(dve_sem, q + 1)
                        nc.tensor.matmul(
                            psum_y[:, g % 7, :],
                            inter_sb[:, st * P:(st + 1) * P],
                            bt_sb[:, j * OJ:(j + 1) * OJ],
                            start=False, stop=True,
                        ).then_inc(pe_sem, 1)
                        g += 1

        @block.vector
        def _(vector):
            # Zero the padded partitions 17..127 of the rank-dim operands so
            # the 128-row LoRA matmul contributes exactly zero there.
            for p0 in range(2 * RANK, P, 32):
                nc.vector.memset(inter_sb[p0:p0 + 32, :], 0.0).then_inc(c_sem, 16)
                nc.vector.memset(bt_sb[p0:p0 + 32, :], 0.0).then_inc(c_sem, 16)
            for q in range(NQ):
                vector.wait_ge(pei_sem, q + 1)
                nc.vector.tensor_copy(
                    inter_sb[0:RANK, q * SQ:(q + 1) * SQ], psum_i[0:RANK, :]
                ).then_inc(dve_sem, 1)

        @block.scalar
        def _(scalar):
            for g in range(NQ * NJ * NT):
                scalar.wait_ge(pe_sem, g + 1)
                if g >= 4:
                    scalar.wait_ge(st_sems[g % 4], 16 * (g // 4))
                nc.scalar.copy(out_sb[:, g % 4, :], psum_y[:, g % 7, :]).then_inc(
                    ev_sem, 1
                )

        @block.gpsimd
        def _(gpsimd):
            for g in range(NQ * NJ * NT - 4):
                q, rem = divmod(g, NJ * NT)
                j, t = divmod(rem, NT)
                st = q * NT + t
                gpsimd.wait_ge(ev_sem, g + 1)
                gpsimd.dma_start(
                    y[st * P:(st + 1) * P, j * OJ:(j + 1) * OJ], out_sb[:, g % 4, :]
                ).then_inc(st_sems[g % 4], 16)

    return nc


_NC_CACHE = {}


def _get_nc():
    if "nc" not in _NC_CACHE:
        _NC_CACHE["nc"] = build_nc()
    return _NC_CACHE["nc"]


def _conv_x(xc):
    return np.ascontiguousarray(xc.astype(np.float32).T).astype(ml_dtypes.bfloat16)


def make_in_maps(x, W, bias, lora_a, lora_b, adapter_indices):
    wt = np.ascontiguousarray(W.astype(np.float32).T).astype(ml_dtypes.bfloat16)
    ones = np.zeros((RANK, S), dtype=ml_dtypes.bfloat16)
    ones[0, :] = 1
    with ThreadPoolExecutor(max_workers=B) as ex:
        xts = list(ex.map(_conv_x, [x[c] for c in range(B)]))
    in_maps = []
    for c in range(B):
        idx = int(adapter_indices[c])
        at = np.ascontiguousarray(lora_a[idx].astype(np.float32).T).astype(
            ml_dtypes.bfloat16)
        bt = np.concatenate(
            [lora_b[idx].astype(np.float32).T, bias.astype(np.float32)[None, :],
             np.zeros((RANK - 1, D), np.float32)],
            axis=0).astype(ml_dtypes.bfloat16)
        in_maps.append({"xt": xts[c], "wt": wt, "at": at, "bt": bt, "ones": ones})
    return in_maps


def kernel(x, W, bias, lora_a, lora_b, adapter_indices):
    nc = _get_nc()
    in_maps = make_in_maps(x, W, bias, lora_a, lora_b, adapter_indices)
    res = run_bass_kernel_spmd(nc, in_maps, list(range(B)))
    out = np.stack([res.results[c]["y"] for c in range(B)], axis=0)
    return out.astype(np.float32)
